# revision 30
# baseline (speedup 1.0000x reference)
"""Trainium2 Bass kernel for nn_Kernel_6199962355332830965 (sparse_attention).

Reference computation (per batch n, with C=128, H=W=48, HW=2304):
    t1  = max_c x                                (H,W)
    t4  = max(-x, roll(-x, 1, w))                = -min(x, roll_w(x))
    t5  = p5_w * t4
    t6  = w6 @ t5                                (1x1 conv)
    t7  = t6^T x / sqrt(C)                       (HW,HW) score tensor
    t8  = dilated 1x5 conv(t1)                   (C/2,H,W)
    t9  = unfold rows {-2,0,+2}                  (3C/2,H,W)
    t10 = w10 @ t9
    t11 = roll(t10, 1, channels)
    t12 = max(x, t11)
    out = (t7^T t12^T)^T / sqrt(HW)              -> (C,H,W)

Algebraic restructuring used here (validated to ~3e-7 rel err):
  1. t7 is only used bilinearly:  out = (t12 @ t6^T) @ x * s  with
     s = 1/(sqrt(C)*sqrt(HW)).  This collapses the O(HW^2 C) attention
     into two C x C contractions: B[c',c] = sum_p t6[c',p] t12[c,p],
     out[c,q] = sum_c' B[c',c] x[c',q] * s.
  2. conv -> unfold -> 1x1 conv -> channel-roll is linear in t1 and folds
     into a single 15-tap kernel: K_eff[(k,j),o] = sum_m w8[m,j]*w10r[o,3m+k]
     with w10r = roll(w10, 1, axis=0); then
     t11[o,hw] = sum_kj K_eff[(k,j),o] * t1[h+2(k-1), w+3(j-2)] (0-padded).

Sharding: pure batch parallel, 2 batches per core on 8 cores.
"""

import os
import sys

import numpy as np

for _p in ("/opt/trn_rl_repo", "/root/.axon_site/_ro/trn_rl_repo"):
    if os.path.isdir(_p) and _p not in sys.path:
        sys.path.append(_p)

import concourse.bass as bass
import concourse.tile as tile
from concourse import bacc, masks, mybir
from concourse.bass_utils import run_bass_kernel_spmd

N, C, H, W = 16, 128, 48, 48
HW = H * W
NCORES = 8
NB = N // NCORES  # batches per core
SCALE = float(1.0 / (np.sqrt(np.float32(C)) * np.sqrt(np.float32(HW))))

F32 = mybir.dt.float32
F32R = mybir.dt.float32r
BF16 = mybir.dt.bfloat16

# 512-column output chunks (PSUM bank width in fp32)
CHUNKS512 = [(c0, min(512, HW - c0)) for c0 in range(0, HW, 512)]

USE_BF16 = True
OUT_F32R = False  # fp32r final matmul corrupts results on HW (unrounded operands)


def _mm(ap):
    """Matmul operand passthrough (bf16 stays bf16, f32 stays f32)."""
    return ap


def build_kernel(tc, out_d, x_d, p5_d, w6_d, w8_d, w10_d, use_bf16=USE_BF16):
    nc = tc.nc
    DT = BF16 if use_bf16 else F32

    def pe_transpose(psum_ap, sbuf_ap, ident):
        nc.tensor.transpose(psum_ap, sbuf_ap, ident)

    with (
        tc.tile_pool(name="const", bufs=1) as cpool,
        tc.tile_pool(name="prep", bufs=1) as prep,
        tc.tile_pool(name="batch", bufs=2 if use_bf16 else 1) as bpool,
        tc.tile_pool(name="ps_big", bufs=4, space="PSUM") as ps_big,
        tc.tile_pool(name="ps_small", bufs=2, space="PSUM") as ps_small,
        tc.tile_pool(name="ps_B", bufs=2, space="PSUM") as ps_b,
    ):
        # ---------------- weight prep (once; replicated across batches) ---
        ident = cpool.tile([128, 128], F32, tag="ident")
        masks.make_identity(nc, ident[:])
        if use_bf16:
            ident_dt = cpool.tile([128, 128], BF16, tag="identbf")
            masks.make_identity(nc, ident_dt[:])
        else:
            ident_dt = ident

        # w6T[c, o] = w6[o, c]
        w6_sb = prep.tile([C, C], F32, tag="w6sb")
        nc.sync.dma_start(w6_sb[:], w6_d.ap())
        ps = ps_small.tile([C, C], F32, tag="small")
        pe_transpose(ps[:], w6_sb[:], ident)
        w6T = cpool.tile([C, C], DT, tag="w6T")
        nc.vector.tensor_copy(w6T[:], ps[:])

        # w10T_k[m, o] = w10[(o-1)%C, 3m+k]  (channel roll folded into weights)
        w10_sb = prep.tile([C, 3 * C // 2], F32, tag="w10sb")
        nc.sync.dma_start(w10_sb[:], w10_d.ap())
        w10v = w10_sb[:].rearrange("c (m k) -> c k m", k=3)
        w8_sb = prep.tile([C // 2, 5], F32, tag="w8sb")
        nc.sync.dma_start(w8_sb[:], w8_d.ap()[:, 0, 0, :])

        pk = ps_small.tile([5, 3 * C], F32, tag="small")
        for k in range(3):
            psk = ps_small.tile([C // 2, C], F32, tag="small")
            pe_transpose(psk[:], w10v[:, k, :], ident)
            w10Tk = prep.tile([C // 2, C], F32, tag=f"w10T{k}")
            # roll output channel: w10Tk[:, o] = (transpose)[:, o-1 mod C]
            nc.vector.tensor_copy(w10Tk[:, 1:C], psk[:, 0 : C - 1])
            nc.vector.tensor_copy(w10Tk[:, 0:1], psk[:, C - 1 : C])
            # K_eff^T[(k,j), o] = sum_m w8[m, j] * w10Tk[m, o]
            nc.tensor.matmul(
                pk[:, k * C : (k + 1) * C],
                w8_sb[:],
                w10Tk[:],
                start=True,
                stop=True,
            )
        keff_tmp = prep.tile([5, 3 * C], DT, tag="kefftmp")
        nc.vector.tensor_copy(keff_tmp[:], pk[:])
        # K_effT row order is (3j + k): one DMA pairing (j, k, o) -> row 3j+k
        K_effT = cpool.tile([15, C], DT, tag="KeffT")
        nc.sync.dma_start(
            K_effT[:], keff_tmp[:].rearrange("j (k o) -> j k o", k=3)
        )

        # neg_p5 = -p5_w  (so t5 = neg_p5 * min(x, roll_w(x)))
        p5_sb = prep.tile([C, HW], F32, tag="p5sb")
        nc.sync.dma_start(p5_sb[:], p5_d.ap()[0].rearrange("c h w -> c (h w)"))
        neg_p5 = cpool.tile([C, HW], DT, tag="negp5")
        nc.vector.tensor_scalar_mul(neg_p5[:], p5_sb[:], -1.0)

        # zero-padded t1 scratch rows: layout (1, 52, 60) with interior
        # [2:50, 6:54]; pads zeroed once, interiors rewritten per batch.
        t1p_tiles = []
        for b in range(NB):
            tp = cpool.tile([1, 52 * 60], DT, tag=f"t1p{b}")
            v = tp[:].rearrange("p (r c) -> p r c", c=60)
            nc.gpsimd.memset(v[:, 0:2, :], 0.0)
            nc.gpsimd.memset(v[:, 50:52, :], 0.0)
            nc.gpsimd.memset(v[:, 2:50, 0:6], 0.0)
            nc.gpsimd.memset(v[:, 2:50, 54:60], 0.0)
            t1p_tiles.append(tp)

        # ---------------- per-batch pipeline ------------------------------
        # Phase-interleaved across the NB batches so the PE always has
        # ready matmul work while one batch's t1->shifted DMA chain drains
        # (keeps the HAM activity monitor at the 2.4 GHz clock).
        x_bf = []
        for b in range(NB):
            # single load of x, cast f32->bf16 in the DMA (SWDGE cast, free)
            xb = bpool.tile([C, HW], DT, tag="x")
            if use_bf16:
                nc.gpsimd.dma_start(
                    xb[:], x_d.ap()[b].rearrange("c h w -> c (h w)")
                )
            else:
                nc.sync.dma_start(xb[:], x_d.ap()[b].rearrange("c h w -> c (h w)"))
            x_bf.append(xb)

        # xT transposes + channel-max reduce
        xT_l, t1pk_l = [], []
        for b in range(NB):
            xT_sb = bpool.tile([C, HW], DT, tag="xT")
            t1pk = bpool.tile([C, 18], F32, tag="t1pk")
            for c0, cn in CHUNKS512:
                psx = ps_big.tile([C, 512], F32, tag="ps")
                pxv = psx[:].bitcast(DT) if use_bf16 else psx[:]
                g = cn // 128
                for j in range(g):
                    col = c0 + j * 128
                    pe_transpose(
                        pxv[:, j * 128 : (j + 1) * 128],
                        x_bf[b][:, col : col + 128],
                        ident_dt,
                    )
                nc.scalar.copy(xT_sb[:, c0 : c0 + cn], pxv[:, :cn])
                nc.vector.reduce_max(
                    t1pk[:, c0 // 128 : c0 // 128 + g],
                    xT_sb[:, c0 : c0 + cn].rearrange("p (g q) -> p g q", q=128),
                    axis=mybir.AxisListType.X,
                )
            xT_l.append(xT_sb)
            t1pk_l.append(t1pk)

        # t1 row -> padded -> replicated -> shifted (DMA latency chain)
        shifted_l = []
        for b in range(NB):
            pst = ps_small.tile([18, C], F32, tag="small")
            pe_transpose(pst[:], t1pk_l[b][:], ident)
            t1row = bpool.tile([18, C], DT, tag="t1row")
            nc.vector.tensor_copy(t1row[:], pst[:])
            tp = t1p_tiles[b]
            v = tp[:].rearrange("p (r c) -> p r c", c=60)
            t1flat = bpool.tile([1, HW], DT, tag="t1flat")
            nc.sync.dma_start(t1flat[:], t1row[:])
            nc.sync.dma_start(v[:, 2:50, 6:54], t1flat[:])
            t1p3 = bpool.tile([3, 2880], DT, tag="t1p3")
            tp_ap = tp[:]
            rep_src = bass.AP(
                tp_ap.tensor,
                tp_ap.offset,
                [list(tp_ap.ap[0]), [120, 3], [1, 2880]],
            )
            nc.sync.dma_start(t1p3[:], rep_src)
            shifted = bpool.tile([15, HW], DT, tag="shifted")
            t1p3v = t1p3[:].rearrange("k (r c) -> k r c", c=60)
            for j in range(5):
                nc.sync.dma_start(
                    shifted[3 * j : 3 * j + 3, :],
                    t1p3v[:, 0:48, 3 * j : 3 * j + 48],
                )
            shifted_l.append(shifted)

        # t5 = neg_p5 * min(x, roll_w(x)), per 512-chunk for fine overlap
        t5_l = []
        for b in range(NB):
            xv = x_bf[b]
            x3 = xv[:].rearrange("c (h w) -> c h w", w=W)
            tmin = bpool.tile([C, HW], DT, tag="tmin")
            tm3 = tmin[:].rearrange("c (h w) -> c h w", w=W)
            nc.vector.tensor_tensor(
                tm3[:, :, 1:W], x3[:, :, 1:W], x3[:, :, 0 : W - 1], mybir.AluOpType.min
            )
            nc.vector.tensor_tensor(
                tm3[:, :, 0:1], x3[:, :, 0:1], x3[:, :, W - 1 : W], mybir.AluOpType.min
            )
            t5 = bpool.tile([C, HW], DT, tag="t5")
            for c0, cn in CHUNKS512:
                nc.vector.tensor_tensor(
                    t5[:, c0 : c0 + cn],
                    tmin[:, c0 : c0 + cn],
                    neg_p5[:, c0 : c0 + cn],
                    mybir.AluOpType.mult,
                )
            t5_l.append(t5)

        # t6T chunks: t6T[p, o] = sum_c t5[c, p] * w6T[c, o]
        t6T_l = []
        for b in range(NB):
            t6T = bpool.tile([C, HW], DT, tag="t6T")
            for c0, cn in CHUNKS512:
                ps6 = ps_big.tile([C, 512], F32, tag="ps")
                for j in range(cn // 128):
                    col = c0 + j * 128
                    nc.tensor.matmul(
                        ps6[:, j * 128 : (j + 1) * 128],
                        t5_l[b][:, col : col + 128],
                        w6T[:],
                        start=True,
                        stop=True,
                    )
                nc.scalar.copy(t6T[:, c0 : c0 + cn], ps6[:, :cn])
            t6T_l.append(t6T)

        # t12T = max(xT, t11T)
        t12T_l = []
        for b in range(NB):
            t12T = bpool.tile([C, HW], DT, tag="t12T")
            for c0, cn in CHUNKS512:
                ps11 = ps_big.tile([C, 512], F32, tag="ps")
                for j in range(cn // 128):
                    col = c0 + j * 128
                    nc.tensor.matmul(
                        ps11[:, j * 128 : (j + 1) * 128],
                        shifted_l[b][:, col : col + 128],
                        K_effT[:],
                        start=True,
                        stop=True,
                    )
                nc.vector.tensor_tensor(
                    t12T[:, c0 : c0 + cn],
                    xT_l[b][:, c0 : c0 + cn],
                    ps11[:, :cn],
                    mybir.AluOpType.max,
                )
            t12T_l.append(t12T)

        # B[c', c] = sum_p t6[c', p] * t12[c, p]; out = s * B^T @ x
        for b in range(NB):
            psB = ps_b.tile([C, C], F32, tag="B")
            for i in range(18):
                nc.tensor.matmul(
                    psB[:],
                    t6T_l[b][:, i * 128 : (i + 1) * 128],
                    t12T_l[b][:, i * 128 : (i + 1) * 128],
                    start=(i == 0),
                    stop=(i == 17),
                )
            Bs = bpool.tile([C, C], DT, tag="Bs")
            nc.scalar.mul(Bs[:], psB[:], SCALE)

            out_sb = bpool.tile([C, HW], F32, tag="osb")
            out_ap = out_d.ap()[b].rearrange("c h w -> c (h w)")
            # drain copies split V/S by batch; out DMAs ride the idle
            # SWDGE queue as half-batch transfers
            drain = nc.vector.tensor_copy if b == 0 else nc.scalar.copy
            for c0, cn in CHUNKS512:
                pso = ps_big.tile([C, 512], F32, tag="ps")
                nc.tensor.matmul(
                    pso[:, :cn],
                    Bs[:],
                    x_bf[b][:, c0 : c0 + cn],
                    start=True,
                    stop=True,
                )
                drain(out_sb[:, c0 : c0 + cn], pso[:, :cn])
            for h0, hn in ((0, HW // 2), (HW // 2, HW // 2)):
                nc.gpsimd.dma_start(
                    out_ap[:, h0 : h0 + hn], out_sb[:, h0 : h0 + hn]
                )


def build_bass(use_bf16=USE_BF16):
    nc = bacc.Bacc("TRN2", target_bir_lowering=False, debug=False, num_devices=NCORES)
    x_d = nc.dram_tensor("x", [NB, C, H, W], F32, kind="ExternalInput")
    p5_d = nc.dram_tensor("p5_w", [1, C, H, W], F32, kind="ExternalInput")
    w6_d = nc.dram_tensor("w6", [C, C], F32, kind="ExternalInput")
    w8_d = nc.dram_tensor("w8", [C // 2, 1, 1, 5], F32, kind="ExternalInput")
    w10_d = nc.dram_tensor("w10", [C, 3 * C // 2], F32, kind="ExternalInput")
    out_d = nc.dram_tensor("out", [NB, C, H, W], F32, kind="ExternalOutput")
    with tile.TileContext(nc) as tc:
        build_kernel(tc, out_d, x_d, p5_d, w6_d, w8_d, w10_d, use_bf16)
    nc.compile()
    return nc


_NC_CACHE = {}


def _get_nc(use_bf16=USE_BF16):
    if use_bf16 not in _NC_CACHE:
        _NC_CACHE[use_bf16] = build_bass(use_bf16)
    return _NC_CACHE[use_bf16]


def kernel(x, p5_w, w6, w8, w10, trace=False, trace_kwargs=None):
    x = np.ascontiguousarray(x, dtype=np.float32)
    nc = _get_nc()
    in_maps = []
    for core in range(NCORES):
        in_maps.append(
            {
                "x": x[core * NB : (core + 1) * NB],
                "p5_w": np.asarray(p5_w, dtype=np.float32),
                "w6": np.asarray(w6, dtype=np.float32),
                "w8": np.asarray(w8, dtype=np.float32),
                "w10": np.asarray(w10, dtype=np.float32),
            }
        )
    res = run_bass_kernel_spmd(
        nc,
        in_maps,
        list(range(NCORES)),
        trace=trace,
        **(trace_kwargs or {}),
    )
    out = np.concatenate([res.results[i]["out"] for i in range(NCORES)], axis=0)
    if trace:
        return out, res
    return out



# revision 31
# speedup vs baseline: 1.0174x; 1.0174x over previous
"""Trainium2 Bass kernel for nn_Kernel_6199962355332830965 (sparse_attention).

Reference computation (per batch n, with C=128, H=W=48, HW=2304):
    t1  = max_c x                                (H,W)
    t4  = max(-x, roll(-x, 1, w))                = -min(x, roll_w(x))
    t5  = p5_w * t4
    t6  = w6 @ t5                                (1x1 conv)
    t7  = t6^T x / sqrt(C)                       (HW,HW) score tensor
    t8  = dilated 1x5 conv(t1)                   (C/2,H,W)
    t9  = unfold rows {-2,0,+2}                  (3C/2,H,W)
    t10 = w10 @ t9
    t11 = roll(t10, 1, channels)
    t12 = max(x, t11)
    out = (t7^T t12^T)^T / sqrt(HW)              -> (C,H,W)

Algebraic restructuring used here (validated to ~3e-7 rel err):
  1. t7 is only used bilinearly:  out = (t12 @ t6^T) @ x * s  with
     s = 1/(sqrt(C)*sqrt(HW)).  This collapses the O(HW^2 C) attention
     into two C x C contractions: B[c',c] = sum_p t6[c',p] t12[c,p],
     out[c,q] = sum_c' B[c',c] x[c',q] * s.
  2. conv -> unfold -> 1x1 conv -> channel-roll is linear in t1 and folds
     into a single 15-tap kernel: K_eff[(k,j),o] = sum_m w8[m,j]*w10r[o,3m+k]
     with w10r = roll(w10, 1, axis=0); then
     t11[o,hw] = sum_kj K_eff[(k,j),o] * t1[h+2(k-1), w+3(j-2)] (0-padded).

Sharding: pure batch parallel, 2 batches per core on 8 cores.
"""

import os
import sys

import numpy as np

for _p in ("/opt/trn_rl_repo", "/root/.axon_site/_ro/trn_rl_repo"):
    if os.path.isdir(_p) and _p not in sys.path:
        sys.path.append(_p)

import concourse.bass as bass
import concourse.tile as tile
from concourse import bacc, masks, mybir
from concourse.bass_utils import run_bass_kernel_spmd

N, C, H, W = 16, 128, 48, 48
HW = H * W
NCORES = 8
NB = N // NCORES  # batches per core
SCALE = float(1.0 / (np.sqrt(np.float32(C)) * np.sqrt(np.float32(HW))))

F32 = mybir.dt.float32
F32R = mybir.dt.float32r
BF16 = mybir.dt.bfloat16

# 512-column output chunks (PSUM bank width in fp32)
CHUNKS512 = [(c0, min(512, HW - c0)) for c0 in range(0, HW, 512)]

USE_BF16 = True
OUT_F32R = False  # fp32r final matmul corrupts results on HW (unrounded operands)


def _mm(ap):
    """Matmul operand passthrough (bf16 stays bf16, f32 stays f32)."""
    return ap


def build_kernel(tc, out_d, x_d, p5_d, w6_d, w8_d, w10_d, use_bf16=USE_BF16):
    nc = tc.nc
    DT = BF16 if use_bf16 else F32

    def pe_transpose(psum_ap, sbuf_ap, ident):
        nc.tensor.transpose(psum_ap, sbuf_ap, ident)

    with (
        tc.tile_pool(name="const", bufs=1) as cpool,
        tc.tile_pool(name="prep", bufs=1) as prep,
        tc.tile_pool(name="batch", bufs=2 if use_bf16 else 1) as bpool,
        tc.tile_pool(name="ps_big", bufs=4, space="PSUM") as ps_big,
        tc.tile_pool(name="ps_small", bufs=2, space="PSUM") as ps_small,
        tc.tile_pool(name="ps_B", bufs=2, space="PSUM") as ps_b,
    ):
        # ---------------- weight prep (once; replicated across batches) ---
        ident = cpool.tile([128, 128], F32, tag="ident")
        masks.make_identity(nc, ident[:])
        if use_bf16:
            ident_dt = cpool.tile([128, 128], BF16, tag="identbf")
            masks.make_identity(nc, ident_dt[:])
        else:
            ident_dt = ident

        # w6T[c, o] = w6[o, c]
        w6_sb = prep.tile([C, C], F32, tag="w6sb")
        nc.sync.dma_start(w6_sb[:], w6_d.ap())
        ps = ps_small.tile([C, C], F32, tag="small")
        pe_transpose(ps[:], w6_sb[:], ident)
        w6T = cpool.tile([C, C], DT, tag="w6T")
        nc.vector.tensor_copy(w6T[:], ps[:])

        # w10T_k[m, o] = w10[(o-1)%C, 3m+k]  (channel roll folded into weights)
        w10_sb = prep.tile([C, 3 * C // 2], F32, tag="w10sb")
        nc.sync.dma_start(w10_sb[:], w10_d.ap())
        w10v = w10_sb[:].rearrange("c (m k) -> c k m", k=3)
        w8_sb = prep.tile([C // 2, 5], F32, tag="w8sb")
        nc.sync.dma_start(w8_sb[:], w8_d.ap()[:, 0, 0, :])

        pk = ps_small.tile([5, 3 * C], F32, tag="small")
        for k in range(3):
            psk = ps_small.tile([C // 2, C], F32, tag="small")
            pe_transpose(psk[:], w10v[:, k, :], ident)
            w10Tk = prep.tile([C // 2, C], F32, tag=f"w10T{k}")
            # roll output channel: w10Tk[:, o] = (transpose)[:, o-1 mod C]
            nc.vector.tensor_copy(w10Tk[:, 1:C], psk[:, 0 : C - 1])
            nc.vector.tensor_copy(w10Tk[:, 0:1], psk[:, C - 1 : C])
            # K_eff^T[(k,j), o] = sum_m w8[m, j] * w10Tk[m, o]
            nc.tensor.matmul(
                pk[:, k * C : (k + 1) * C],
                w8_sb[:],
                w10Tk[:],
                start=True,
                stop=True,
            )
        keff_tmp = prep.tile([5, 3 * C], DT, tag="kefftmp")
        nc.vector.tensor_copy(keff_tmp[:], pk[:])
        # K_effT row order is (3j + k): one DMA pairing (j, k, o) -> row 3j+k
        K_effT = cpool.tile([15, C], DT, tag="KeffT")
        nc.sync.dma_start(
            K_effT[:], keff_tmp[:].rearrange("j (k o) -> j k o", k=3)
        )

        # neg_p5 = -p5_w  (so t5 = neg_p5 * min(x, roll_w(x)))
        p5_sb = prep.tile([C, HW], F32, tag="p5sb")
        nc.sync.dma_start(p5_sb[:], p5_d.ap()[0].rearrange("c h w -> c (h w)"))
        neg_p5 = cpool.tile([C, HW], DT, tag="negp5")
        nc.vector.tensor_scalar_mul(neg_p5[:], p5_sb[:], -1.0)

        # zero-padded t1 scratch rows: layout (1, 52, 60) with interior
        # [2:50, 6:54]; pads zeroed once, interiors rewritten per batch.
        t1p_tiles = []
        for b in range(NB):
            tp = cpool.tile([1, 52 * 60], DT, tag=f"t1p{b}")
            v = tp[:].rearrange("p (r c) -> p r c", c=60)
            nc.gpsimd.memset(v[:, 0:2, :], 0.0)
            nc.gpsimd.memset(v[:, 50:52, :], 0.0)
            nc.gpsimd.memset(v[:, 2:50, 0:6], 0.0)
            nc.gpsimd.memset(v[:, 2:50, 54:60], 0.0)
            t1p_tiles.append(tp)

        # ---------------- per-batch pipeline ------------------------------
        # Phase-interleaved across the NB batches so the PE always has
        # ready matmul work while one batch's t1->shifted DMA chain drains
        # (keeps the HAM activity monitor at the 2.4 GHz clock).
        x_bf = []
        for b in range(NB):
            # single load of x, cast f32->bf16 in the DMA (SWDGE cast, free)
            xb = bpool.tile([C, HW], DT, tag="x")
            if use_bf16:
                nc.gpsimd.dma_start(
                    xb[:], x_d.ap()[b].rearrange("c h w -> c (h w)")
                )
            else:
                nc.sync.dma_start(xb[:], x_d.ap()[b].rearrange("c h w -> c (h w)"))
            x_bf.append(xb)

        # xT transposes + channel-max reduce
        xT_l, t1pk_l = [], []
        for b in range(NB):
            xT_sb = bpool.tile([C, HW], DT, tag="xT")
            t1pk = bpool.tile([C, 18], F32, tag="t1pk")
            for c0, cn in CHUNKS512:
                psx = ps_big.tile([C, 512], F32, tag="ps")
                pxv = psx[:].bitcast(DT) if use_bf16 else psx[:]
                g = cn // 128
                for j in range(g):
                    col = c0 + j * 128
                    pe_transpose(
                        pxv[:, j * 128 : (j + 1) * 128],
                        x_bf[b][:, col : col + 128],
                        ident_dt,
                    )
                nc.scalar.copy(xT_sb[:, c0 : c0 + cn], pxv[:, :cn])
                nc.vector.reduce_max(
                    t1pk[:, c0 // 128 : c0 // 128 + g],
                    xT_sb[:, c0 : c0 + cn].rearrange("p (g q) -> p g q", q=128),
                    axis=mybir.AxisListType.X,
                )
            xT_l.append(xT_sb)
            t1pk_l.append(t1pk)

        # t1 row -> padded -> replicated -> shifted (DMA latency chain)
        shifted_l = []
        for b in range(NB):
            pst = ps_small.tile([18, C], F32, tag="small")
            pe_transpose(pst[:], t1pk_l[b][:], ident)
            t1row = bpool.tile([18, C], DT, tag="t1row")
            nc.vector.tensor_copy(t1row[:], pst[:])
            tp = t1p_tiles[b]
            v = tp[:].rearrange("p (r c) -> p r c", c=60)
            t1flat = bpool.tile([1, HW], DT, tag="t1flat")
            nc.sync.dma_start(t1flat[:], t1row[:])
            nc.sync.dma_start(v[:, 2:50, 6:54], t1flat[:])
            t1p3 = bpool.tile([3, 2880], DT, tag="t1p3")
            tp_ap = tp[:]
            rep_src = bass.AP(
                tp_ap.tensor,
                tp_ap.offset,
                [list(tp_ap.ap[0]), [120, 3], [1, 2880]],
            )
            nc.sync.dma_start(t1p3[:], rep_src)
            shifted = bpool.tile([15, HW], DT, tag="shifted")
            t1p3v = t1p3[:].rearrange("k (r c) -> k r c", c=60)
            for j in range(5):
                nc.sync.dma_start(
                    shifted[3 * j : 3 * j + 3, :],
                    t1p3v[:, 0:48, 3 * j : 3 * j + 48],
                )
            shifted_l.append(shifted)

        # t5 = neg_p5 * min(x, roll_w(x)), per 512-chunk for fine overlap
        t5_l = []
        for b in range(NB):
            xv = x_bf[b]
            x3 = xv[:].rearrange("c (h w) -> c h w", w=W)
            tmin = bpool.tile([C, HW], DT, tag="tmin")
            tm3 = tmin[:].rearrange("c (h w) -> c h w", w=W)
            nc.vector.tensor_tensor(
                tm3[:, :, 1:W], x3[:, :, 1:W], x3[:, :, 0 : W - 1], mybir.AluOpType.min
            )
            nc.vector.tensor_tensor(
                tm3[:, :, 0:1], x3[:, :, 0:1], x3[:, :, W - 1 : W], mybir.AluOpType.min
            )
            t5 = bpool.tile([C, HW], DT, tag="t5")
            for c0, cn in CHUNKS512:
                nc.vector.tensor_tensor(
                    t5[:, c0 : c0 + cn],
                    tmin[:, c0 : c0 + cn],
                    neg_p5[:, c0 : c0 + cn],
                    mybir.AluOpType.mult,
                )
            t5_l.append(t5)

        # t6T chunks: t6T[p, o] = sum_c t5[c, p] * w6T[c, o]
        t6T_l = []
        for b in range(NB):
            t6T = bpool.tile([C, HW], DT, tag="t6T")
            for c0, cn in CHUNKS512:
                ps6 = ps_big.tile([C, 512], F32, tag="ps")
                for j in range(cn // 128):
                    col = c0 + j * 128
                    nc.tensor.matmul(
                        ps6[:, j * 128 : (j + 1) * 128],
                        t5_l[b][:, col : col + 128],
                        w6T[:],
                        start=True,
                        stop=True,
                    )
                nc.scalar.copy(t6T[:, c0 : c0 + cn], ps6[:, :cn])
            t6T_l.append(t6T)

        # t12T = max(xT, t11T)
        t12T_l = []
        for b in range(NB):
            t12T = bpool.tile([C, HW], DT, tag="t12T")
            for c0, cn in CHUNKS512:
                ps11 = ps_big.tile([C, 512], F32, tag="ps")
                for j in range(cn // 128):
                    col = c0 + j * 128
                    nc.tensor.matmul(
                        ps11[:, j * 128 : (j + 1) * 128],
                        shifted_l[b][:, col : col + 128],
                        K_effT[:],
                        start=True,
                        stop=True,
                    )
                nc.vector.tensor_tensor(
                    t12T[:, c0 : c0 + cn],
                    xT_l[b][:, c0 : c0 + cn],
                    ps11[:, :cn],
                    mybir.AluOpType.max,
                )
            t12T_l.append(t12T)

        # B[c', c] = sum_p t6[c', p] * t12[c, p]; out = s * B^T @ x
        for b in range(NB):
            psB = ps_b.tile([C, C], F32, tag="B")
            for i in range(18):
                nc.tensor.matmul(
                    psB[:],
                    t6T_l[b][:, i * 128 : (i + 1) * 128],
                    t12T_l[b][:, i * 128 : (i + 1) * 128],
                    start=(i == 0),
                    stop=(i == 17),
                )
            Bs = bpool.tile([C, C], DT, tag="Bs")
            nc.scalar.mul(Bs[:], psB[:], SCALE)

            out_sb = bpool.tile([C, HW], F32, tag="osb")
            out_ap = out_d.ap()[b].rearrange("c h w -> c (h w)")
            for c0, cn in CHUNKS512:
                pso = ps_big.tile([C, 512], F32, tag="ps")
                nc.tensor.matmul(
                    pso[:, :cn],
                    Bs[:],
                    x_bf[b][:, c0 : c0 + cn],
                    start=True,
                    stop=True,
                )
                nc.scalar.copy(out_sb[:, c0 : c0 + cn], pso[:, :cn])
                nc.sync.dma_start(out_ap[:, c0 : c0 + cn], out_sb[:, c0 : c0 + cn])


def build_bass(use_bf16=USE_BF16):
    nc = bacc.Bacc("TRN2", target_bir_lowering=False, debug=False, num_devices=NCORES)
    x_d = nc.dram_tensor("x", [NB, C, H, W], F32, kind="ExternalInput")
    p5_d = nc.dram_tensor("p5_w", [1, C, H, W], F32, kind="ExternalInput")
    w6_d = nc.dram_tensor("w6", [C, C], F32, kind="ExternalInput")
    w8_d = nc.dram_tensor("w8", [C // 2, 1, 1, 5], F32, kind="ExternalInput")
    w10_d = nc.dram_tensor("w10", [C, 3 * C // 2], F32, kind="ExternalInput")
    out_d = nc.dram_tensor("out", [NB, C, H, W], F32, kind="ExternalOutput")
    with tile.TileContext(nc) as tc:
        build_kernel(tc, out_d, x_d, p5_d, w6_d, w8_d, w10_d, use_bf16)
    nc.compile()
    return nc


_NC_CACHE = {}


def _get_nc(use_bf16=USE_BF16):
    if use_bf16 not in _NC_CACHE:
        _NC_CACHE[use_bf16] = build_bass(use_bf16)
    return _NC_CACHE[use_bf16]


def kernel(x, p5_w, w6, w8, w10, trace=False, trace_kwargs=None):
    x = np.ascontiguousarray(x, dtype=np.float32)
    nc = _get_nc()
    in_maps = []
    for core in range(NCORES):
        in_maps.append(
            {
                "x": x[core * NB : (core + 1) * NB],
                "p5_w": np.asarray(p5_w, dtype=np.float32),
                "w6": np.asarray(w6, dtype=np.float32),
                "w8": np.asarray(w8, dtype=np.float32),
                "w10": np.asarray(w10, dtype=np.float32),
            }
        )
    res = run_bass_kernel_spmd(
        nc,
        in_maps,
        list(range(NCORES)),
        trace=trace,
        **(trace_kwargs or {}),
    )
    out = np.concatenate([res.results[i]["out"] for i in range(NCORES)], axis=0)
    if trace:
        return out, res
    return out

